# revision 8
# baseline (speedup 1.0000x reference)
"""CapsuleLayer dynamic-routing kernel for 8 Trainium2 NeuronCores.

Problem: x[32, 2048, 16], W[1, 2048, 64, 32, 16] -> v[32, 64, 32]
  u_hat = einsum('iodk,bik->biod', W[0], x)
  3 routing iterations (softmax over out_caps, squash over out_dim).

Sharding: in_caps (i) split 8 ways (256/core), W shard SBUF-resident bf16.

v2 layout: u_hat columns are (d, o)-ordered [col = d*64 + o].  This makes
every hot DVE op eligible for the 2x/4x perf modes:
  - V-mul: plain bf16*bf16 tensor_tensor (2x)
  - agreement d-reduce: 5 contiguous halving adds (bf16 2x) instead of a
    1x tensor_reduce
  - c-apply: rZ folded into eB ([128,64] tensor_scalar, 4x), then ONE
    tensor_tensor with eB broadcast on the OUTER (d) dim so the innermost
    AP stays packed (2x)
PSUM: u_hat quad is built in two [128,1024] halves (2 banks each, double
buffered) + s accumulator [32,2048] (4 banks) = 8 banks total, so the PE
can run ahead of the scalar-engine evacuation.

Routing state trick: b_ij(t) = sum_d u_hat * (v_0+...+v_{t-1}), so no
b_ij state is carried - only the accumulated V (bf16 [128,2048], x4
partition-group replication).
"""

import numpy as np
import ml_dtypes

B, IC, KD, OC, OD = 32, 2048, 16, 64, 32     # batch, in_caps, in_dim, out_caps, out_dim
NCORES = 8
ICC = IC // NCORES                            # 256 in_caps per core
NJ = ICC // 8                                 # 32 j-blocks (8 i per block)
OD2 = OC * OD                                 # 2048 flattened (d, o) columns
NUM_ROUTES = 3

_CACHE = {}


def _build_program():
    import concourse.bacc as bacc
    import concourse.tile as tile
    import concourse.mybir as mybir

    f32 = mybir.dt.float32
    bf16 = mybir.dt.bfloat16
    ALU = mybir.AluOpType
    ACTF = mybir.ActivationFunctionType

    nc = bacc.Bacc("TRN2", target_bir_lowering=False, debug=False, num_devices=NCORES)

    WL_d = nc.dram_tensor("WL", [128, NJ * OD2], bf16, kind="ExternalInput").ap()
    xS0_d = nc.dram_tensor("xS0", [128, NJ * B], bf16, kind="ExternalInput").ap()
    xS1_d = nc.dram_tensor("xS1", [128, NJ * B], bf16, kind="ExternalInput").ap()
    SEL1_d = nc.dram_tensor("SEL1", [128, 32], bf16, kind="ExternalInput").ap()
    X2_d = nc.dram_tensor("X2", [128, NJ * B], bf16, kind="ExternalInput").ap()
    vout_d = nc.dram_tensor("v_out", [B, OD2], f32, kind="ExternalOutput").ap()

    with tile.TileContext(nc) as tc:
        with (
            tc.tile_pool(name="const", bufs=1) as cp,
            tc.tile_pool(name="work", bufs=2) as wp,
            tc.tile_pool(name="small", bufs=2) as sp,
            tc.tile_pool(name="psacc", bufs=1, space="PSUM") as pa,
            tc.tile_pool(name="psuh", bufs=2, space="PSUM") as pu,
            tc.tile_pool(name="dram", bufs=1, space="DRAM") as dp,
        ):
            # ---- resident inputs ----
            wl = cp.tile([128, NJ * OD2], bf16, tag="wl")
            for blk in range(8):
                w = NJ * OD2 // 8
                nc.sync.dma_start(out=wl[:, blk * w:(blk + 1) * w],
                                  in_=WL_d[:, blk * w:(blk + 1) * w])
            xs = [cp.tile([128, NJ * B], bf16, tag=f"xs{s}", name=f"xs{s}") for s in range(2)]
            nc.sync.dma_start(out=xs[0][:, :], in_=xS0_d[:, :])
            nc.sync.dma_start(out=xs[1][:, :], in_=xS1_d[:, :])
            sel1 = cp.tile([128, 32], bf16, tag="sel1")
            nc.sync.dma_start(out=sel1[:, :], in_=SEL1_d[:, :])
            x2t = cp.tile([128, NJ * B], bf16, tag="x2t")
            nc.sync.dma_start(out=x2t[:, :], in_=X2_d[:, :])

            # ---- persistent state ----
            V4 = cp.tile([128, OD2], bf16, tag="V4")   # V replicated x4 part-groups
            Vacc = cp.tile([B, OD2], bf16, tag="Vacc")  # running sum of v_t (bf16)

            ar_in = [dp.tile([B, OD2], f32, tag=f"ari{t}", name=f"ari{t}") for t in range(NUM_ROUTES)]
            ar_out = [dp.tile([B, OD2], f32, tag=f"aro{t}", name=f"aro{t}") for t in range(NUM_ROUTES)]

            def allreduce_s(t, src_psum):
                """Evacuate s (psum [32, 2048]) -> allreduce -> s_sb."""
                s_sb = cp.tile([B, OD2], f32, tag="ssb", name=f"s_sb{t}")
                nc.scalar.copy(s_sb[:, :], src_psum[0:B, :])
                nc.sync.dma_start(out=ar_in[t][:, :], in_=s_sb[:, :])
                nc.gpsimd.collective_compute(
                    "AllReduce", ALU.add,
                    replica_groups=[list(range(NCORES))],
                    ins=[ar_in[t].opt()],
                    outs=[ar_out[t].opt()],
                )
                nc.sync.dma_start(out=s_sb[:, :], in_=ar_out[t][:, :])
                return s_sb

            def squash(t, s_sb):
                """v_t = squash(s_sb), (d,o) layout.  t<2: Vacc += v_t,
                V4 <- bf16 replicate(Vacc).  t==2: DMA v_t to output."""
                sq = wp.tile([B, OD2], f32, tag="sqv", name=f"sq{t}", bufs=1)
                nc.scalar.activation(sq[:, :], s_sb[:, :], ACTF.Square)
                # n2[b, o] = sum_d s^2 : in-place halving tree over outer d
                w = OD2
                for lv in range(5):
                    w //= 2
                    nc.vector.tensor_tensor(out=sq[:, :w], in0=sq[:, :w],
                                            in1=sq[:, w:2 * w], op=ALU.add)
                n2 = sp.tile([B, OC], f32, tag="n2")
                nc.vector.tensor_copy(n2[:, :], sq[:, :OC])
                r0 = sp.tile([B, OC], f32, tag="r0")
                nc.scalar.activation(r0[:, :], n2[:, :], ACTF.Sqrt)
                # Newton polish: n = 0.5 * (r0 + n2 / r0)
                t1 = sp.tile([B, OC], f32, tag="t1")
                nc.vector.reciprocal(t1[:, :], r0[:, :])
                nc.vector.tensor_mul(t1[:, :], t1[:, :], n2[:, :])
                t2 = sp.tile([B, OC], f32, tag="t2")
                nc.vector.tensor_add(t2[:, :], t1[:, :], r0[:, :])
                nn = sp.tile([B, OC], f32, tag="nn")
                nc.vector.tensor_scalar_mul(nn[:, :], t2[:, :], 0.5)   # |s|
                den = sp.tile([B, OC], f32, tag="den")
                nc.vector.tensor_scalar_add(den[:, :], n2[:, :], 1.0)
                rec = sp.tile([B, OC], f32, tag="rec")
                nc.vector.reciprocal(rec[:, :], den[:, :])
                qq = sp.tile([B, OC], f32, tag="qq")
                nc.vector.tensor_mul(qq[:, :], nn[:, :], rec[:, :])  # |s|/(1+|s|^2)
                vt = wp.tile([B, OD2], f32, tag="sqv", name=f"vt{t}", bufs=1)
                # v = s * qq broadcast over the OUTER d dim (innermost o packed)
                nc.vector.tensor_tensor(
                    out=vt[:, :].rearrange("p (d o) -> p d o", o=OC),
                    in0=s_sb[:, :].rearrange("p (d o) -> p d o", o=OC),
                    in1=qq[:, :].unsqueeze(1).broadcast_to([B, OD, OC]),
                    op=ALU.mult)
                if t == NUM_ROUTES - 1:
                    nc.sync.dma_start(out=vout_d[:, :], in_=vt[:, :])
                else:
                    if t == 0:
                        nc.vector.tensor_copy(Vacc[:, :], vt[:, :])
                    else:
                        with nc.allow_low_precision(reason="V accum in bf16"):
                            nc.vector.tensor_add(Vacc[:, :], Vacc[:, :], vt[:, :])
                    for g in range(4):
                        nc.sync.dma_start(out=V4[32 * g:32 * g + 32, :], in_=Vacc[:, :])

            # ======== pass 1: s0 = sum_i u_hat / 64 ========
            sacc = pa.tile([B, OD2], f32, tag="sacc", name="sacc0")
            for tau in range(NJ):
                for ch in range(4):
                    nc.tensor.matmul(
                        sacc[0:B, ch * 512:(ch + 1) * 512],
                        lhsT=x2t[:, tau * B:(tau + 1) * B],
                        rhs=wl[:, tau * OD2 + ch * 512: tau * OD2 + (ch + 1) * 512],
                        start=(tau == 0), stop=(tau == NJ - 1),
                        tile_position=(0, 0))
            s_sb = allreduce_s(0, sacc)
            squash(0, s_sb)

            # ======== passes 2..3: fused agreement/softmax/s ========
            for t in range(1, NUM_ROUTES):
                sacc = pa.tile([B, OD2], f32, tag="sacc", name=f"sacc{t}")
                for q in range(2 * NJ):
                    jj, s_ = divmod(q, 2)
                    uhsb = wp.tile([128, OD2], bf16, tag="uhb", name=f"uhsb{t}_{q}")
                    for h in range(2):
                        uh = pu.tile([128, 1024], f32, tag="uh", name=f"uh{t}_{q}_{h}")
                        for ch in range(2):
                            col0 = jj * OD2 + h * 1024 + ch * 512
                            for r in range(4):
                                nc.tensor.matmul(
                                    uh[32 * r:32 * r + 32, ch * 512:(ch + 1) * 512],
                                    lhsT=xs[s_][32 * r:32 * r + 32, jj * B:(jj + 1) * B],
                                    rhs=wl[32 * r:32 * r + 32, col0:col0 + 512],
                                    start=True, stop=True,
                                    tile_position=(32 * r, 32 * r),
                                )
                        # scalar engine evacuates u_hat (f32->bf16): frees
                        # the PSUM half so the PE starts the next one.
                        nc.scalar.copy(uhsb[:, h * 1024:(h + 1) * 1024], uh[:, :])
                    # agreement: tmp = u_hat * V (2x), then halving tree
                    # over d (all contiguous bf16 2x adds)
                    tmp = wp.tile([128, OD2], bf16, tag="tmp", name=f"tmp{t}_{q}")
                    # V-mul on the (otherwise idle) gpsimd engine
                    nc.gpsimd.tensor_mul(tmp[:, :], uhsb[:, :], V4[:, :])
                    w = OD2
                    with nc.allow_low_precision(reason="bf16 agreement tree"):
                        for lv in range(4):
                            w //= 2
                            nc.vector.tensor_tensor(out=tmp[:, :w], in0=tmp[:, :w],
                                                    in1=tmp[:, w:2 * w], op=ALU.add)
                    agr = sp.tile([128, OC], f32, tag="agr")
                    nc.vector.tensor_tensor(out=agr[:, :], in0=tmp[:, :OC],
                                            in1=tmp[:, OC:2 * OC], op=ALU.add)
                    eB = sp.tile([128, OC], bf16, tag="eB")
                    Zs = sp.tile([128, 1], f32, tag="Zs")
                    nc.scalar.activation(eB[:, :], agr[:, :], ACTF.Exp,
                                         accum_out=Zs[:, :])
                    rZ = sp.tile([128, 1], f32, tag="rZ")
                    nc.vector.reciprocal(rZ[:, :], Zs[:, :])
                    eB2 = sp.tile([128, OC], bf16, tag="eB2")
                    nc.vector.tensor_scalar_mul(eB2[:, :], eB[:, :], rZ[:, :])
                    # c-apply: ONE tensor_tensor, eB2 broadcast over outer d
                    tmp2 = wp.tile([128, OD2], bf16, tag="tmp2b", name=f"tmp2b{t}_{q}")
                    nc.vector.tensor_tensor(
                        out=tmp2[:, :].rearrange("p (d o) -> p d o", o=OC),
                        in0=uhsb[:, :].rearrange("p (d o) -> p d o", o=OC),
                        in1=eB2[:, :].unsqueeze(1).broadcast_to([128, OD, OC]),
                        op=ALU.mult)
                    for ch in range(4):
                        nc.tensor.matmul(
                            sacc[0:B, ch * 512:(ch + 1) * 512], lhsT=sel1[:, :],
                            rhs=tmp2[:, ch * 512:(ch + 1) * 512],
                            start=(q == 0), stop=(q == 2 * NJ - 1),
                            tile_position=(0, 0))
                s_sb = allreduce_s(t, sacc)
                squash(t, s_sb)

    nc.compile()
    return nc


def _host_inputs(x, W):
    """Build per-core input maps (host-side relayout, not device time)."""
    W0 = np.asarray(W)[0]                       # [IC, OC, OD, KD]
    x = np.asarray(x)                           # [B, IC, KD]
    in_maps = []
    sel1 = np.zeros((128, 32), np.float32)
    for p in range(128):
        sel1[p, p % 32] = 1.0
    for c in range(NCORES):
        # W layout: partition 16*i8 + k, col tau*2048 + (d*64 + o)  [(d,o)!]
        Wc = W0[c * ICC:(c + 1) * ICC].reshape(NJ, 8, OC, OD, KD)   # [tau, i8, o, d, k]
        WL = np.ascontiguousarray(Wc.transpose(1, 4, 0, 3, 2)       # [i8, k, tau, d, o]
                                  ).reshape(128, NJ * OD2)
        xc = x[:, c * ICC:(c + 1) * ICC, :].reshape(B, NJ, 8, KD)   # [b, tau, i8, k]
        xss = []
        for s in range(2):
            Xs = np.zeros((4, 2, KD, NJ, B), np.float32)            # [r, s', k, tau, b]
            Xs[:, s] = xc[:, :, s::2].transpose(2, 3, 1, 0)         # [r, k, tau, b]
            xss.append(Xs.reshape(128, NJ * B))
        X2 = (np.ascontiguousarray(xc.transpose(2, 3, 1, 0))        # [i8, k, tau, b]
              .reshape(128, NJ * B) / float(OC))
        in_maps.append({
            "WL": WL.astype(ml_dtypes.bfloat16),
            "xS0": xss[0].astype(ml_dtypes.bfloat16),
            "xS1": xss[1].astype(ml_dtypes.bfloat16),
            "SEL1": sel1.astype(ml_dtypes.bfloat16),
            "X2": X2.astype(ml_dtypes.bfloat16),
        })
    return in_maps


def kernel(x, W, _want_trace=False):
    from concourse.bass_utils import run_bass_kernel_spmd

    if "nc" not in _CACHE:
        _CACHE["nc"] = _build_program()
    nc = _CACHE["nc"]
    in_maps = _host_inputs(x, W)
    res = run_bass_kernel_spmd(nc, in_maps, core_ids=list(range(NCORES)),
                               trace=_want_trace)
    _CACHE["last_result"] = res
    out = np.asarray(res.results[0]["v_out"], np.float32)
    # device output is [B, (d, o)] -> reorder to [B, OC, OD]
    return np.ascontiguousarray(out.reshape(B, OD, OC).transpose(0, 2, 1))


# revision 13
# speedup vs baseline: 1.4170x; 1.4170x over previous
"""CapsuleLayer dynamic-routing kernel for 8 Trainium2 NeuronCores.

Problem: x[32, 2048, 16], W[1, 2048, 64, 32, 16] -> v[32, 64, 32]
  u_hat = einsum('iodk,bik->biod', W[0], x)
  3 routing iterations (softmax over out_caps, squash over out_dim).

Sharding: in_caps (i) split 8 ways (256/core), W shard SBUF-resident bf16.

v2 layout: u_hat columns are (d, o)-ordered [col = d*64 + o].  This makes
every hot DVE op eligible for the 2x/4x perf modes:
  - V-mul: plain bf16*bf16 tensor_tensor (2x)
  - agreement d-reduce: 5 contiguous halving adds (bf16 2x) instead of a
    1x tensor_reduce
  - c-apply: rZ folded into eB ([128,64] tensor_scalar, 4x), then ONE
    tensor_tensor with eB broadcast on the OUTER (d) dim so the innermost
    AP stays packed (2x)
PSUM: u_hat quad is built in two [128,1024] halves (2 banks each, double
buffered) + s accumulator [32,2048] (4 banks) = 8 banks total, so the PE
can run ahead of the scalar-engine evacuation.

Routing state trick: b_ij(t) = sum_d u_hat * (v_0+...+v_{t-1}), so no
b_ij state is carried - only the accumulated V (bf16 [128,2048], x4
partition-group replication).
"""

import numpy as np
import ml_dtypes

B, IC, KD, OC, OD = 32, 2048, 16, 64, 32     # batch, in_caps, in_dim, out_caps, out_dim
NCORES = 8
ICC = IC // NCORES                            # 256 in_caps per core
NJ = ICC // 8                                 # 32 j-blocks (8 i per block)
OD2 = OC * OD                                 # 2048 flattened (d, o) columns
NUM_ROUTES = 3

_CACHE = {}


def _build_program():
    import concourse.bacc as bacc
    import concourse.tile as tile
    import concourse.mybir as mybir

    f32 = mybir.dt.float32
    bf16 = mybir.dt.bfloat16
    ALU = mybir.AluOpType
    ACTF = mybir.ActivationFunctionType

    nc = bacc.Bacc("TRN2", target_bir_lowering=False, debug=False, num_devices=NCORES)

    WL_d = nc.dram_tensor("WL", [128, NJ * OD2], bf16, kind="ExternalInput").ap()
    xS0_d = nc.dram_tensor("xS0", [128, NJ * B], bf16, kind="ExternalInput").ap()
    xS1_d = nc.dram_tensor("xS1", [128, NJ * B], bf16, kind="ExternalInput").ap()
    SEL1_d = nc.dram_tensor("SEL1", [128, 32], bf16, kind="ExternalInput").ap()
    X2_d = nc.dram_tensor("X2", [128, NJ * B], bf16, kind="ExternalInput").ap()
    vout_d = nc.dram_tensor("v_out", [B, OD2], f32, kind="ExternalOutput").ap()

    with tile.TileContext(nc) as tc:
        with (
            tc.tile_pool(name="const", bufs=1) as cp,
            tc.tile_pool(name="work", bufs=2) as wp,
            tc.tile_pool(name="small", bufs=2) as sp,
            tc.tile_pool(name="psacc", bufs=1, space="PSUM") as pa,
            tc.tile_pool(name="psuh", bufs=2, space="PSUM") as pu,
            tc.tile_pool(name="dram", bufs=1, space="DRAM") as dp,
        ):
            # ---- resident inputs (small stationaries first: pass 1 needs
            # x2t before any W chunk is useful) ----
            xs = [cp.tile([128, NJ * B], bf16, tag=f"xs{s}", name=f"xs{s}") for s in range(2)]
            sel1 = cp.tile([128, 32], bf16, tag="sel1")
            x2t = cp.tile([128, NJ * B], bf16, tag="x2t")
            nc.sync.dma_start(out=x2t[:, :], in_=X2_d[:, :])
            nc.sync.dma_start(out=xs[0][:, :], in_=xS0_d[:, :])
            nc.sync.dma_start(out=xs[1][:, :], in_=xS1_d[:, :])
            nc.sync.dma_start(out=sel1[:, :], in_=SEL1_d[:, :])
            wl = cp.tile([128, NJ * OD2], bf16, tag="wl")
            for blk in range(8):
                w = NJ * OD2 // 8
                nc.sync.dma_start(out=wl[:, blk * w:(blk + 1) * w],
                                  in_=WL_d[:, blk * w:(blk + 1) * w])

            # ---- persistent state ----
            V4 = cp.tile([128, OD2], bf16, tag="V4")   # V replicated x4 part-groups
            Vacc = cp.tile([B, OD2], bf16, tag="Vacc")  # running sum of v_t (bf16)

            ar_dt = [bf16, bf16, f32]
            ar_in = [dp.tile([B, OD2], ar_dt[t], tag=f"ari{t}", name=f"ari{t}")
                     for t in range(NUM_ROUTES)]
            ar_out = [dp.tile([B, OD2], ar_dt[t], tag=f"aro{t}", name=f"aro{t}",
                              addr_space="Shared")
                      for t in range(NUM_ROUTES)]

            def allreduce_s(t, src_psum):
                """Evacuate s (psum [32, 2048]) -> allreduce -> s_sb.
                Passes 0,1 run the wire in bf16 (half the bytes)."""
                s_sb = cp.tile([B, OD2], ar_dt[t], tag=f"ssb{t}", name=f"s_sb{t}")
                nc.scalar.copy(s_sb[:, :], src_psum[0:B, :])
                nc.sync.dma_start(out=ar_in[t][:, :], in_=s_sb[:, :])
                nc.gpsimd.collective_compute(
                    "AllReduce", ALU.add,
                    replica_groups=[list(range(NCORES))],
                    ins=[ar_in[t].opt()],
                    outs=[ar_out[t].opt()],
                )
                nc.sync.dma_start(out=s_sb[:, :], in_=ar_out[t][:, :])
                return s_sb

            def squash(t, s_sb):
                """v_t = squash(s_sb), (d,o) layout.  t<2: Vacc += v_t,
                V4 <- bf16 replicate(Vacc).  t==2: DMA v_t to output."""
                sq = wp.tile([B, OD2], f32, tag="sqv", name=f"sq{t}", bufs=1)
                nc.scalar.activation(sq[:, :], s_sb[:, :], ACTF.Square)
                # n2[b, o] = sum_d s^2 : in-place halving tree over outer d
                w = OD2
                for lv in range(5):
                    w //= 2
                    nc.vector.tensor_tensor(out=sq[:, :w], in0=sq[:, :w],
                                            in1=sq[:, w:2 * w], op=ALU.add)
                n2 = sp.tile([B, OC], f32, tag="n2")
                nc.vector.tensor_copy(n2[:, :], sq[:, :OC])
                r0 = sp.tile([B, OC], f32, tag="r0")
                nc.scalar.activation(r0[:, :], n2[:, :], ACTF.Sqrt)
                # Newton polish: n = 0.5 * (r0 + n2 / r0)
                t1 = sp.tile([B, OC], f32, tag="t1")
                nc.vector.reciprocal(t1[:, :], r0[:, :])
                nc.vector.tensor_mul(t1[:, :], t1[:, :], n2[:, :])
                t2 = sp.tile([B, OC], f32, tag="t2")
                nc.vector.tensor_add(t2[:, :], t1[:, :], r0[:, :])
                nn = sp.tile([B, OC], f32, tag="nn")
                nc.vector.tensor_scalar_mul(nn[:, :], t2[:, :], 0.5)   # |s|
                den = sp.tile([B, OC], f32, tag="den")
                nc.vector.tensor_scalar_add(den[:, :], n2[:, :], 1.0)
                rec = sp.tile([B, OC], f32, tag="rec")
                nc.vector.reciprocal(rec[:, :], den[:, :])
                qq = sp.tile([B, OC], f32, tag="qq")
                nc.vector.tensor_mul(qq[:, :], nn[:, :], rec[:, :])  # |s|/(1+|s|^2)
                vt = wp.tile([B, OD2], f32, tag="sqv", name=f"vt{t}", bufs=1)
                # v = s * qq broadcast over the OUTER d dim (innermost o packed)
                nc.vector.tensor_tensor(
                    out=vt[:, :].rearrange("p (d o) -> p d o", o=OC),
                    in0=s_sb[:, :].rearrange("p (d o) -> p d o", o=OC),
                    in1=qq[:, :].unsqueeze(1).broadcast_to([B, OD, OC]),
                    op=ALU.mult)
                if t == NUM_ROUTES - 1:
                    nc.sync.dma_start(out=vout_d[:, :], in_=vt[:, :])
                else:
                    if t == 0:
                        nc.vector.tensor_copy(Vacc[:, :], vt[:, :])
                    else:
                        with nc.allow_low_precision(reason="V accum in bf16"):
                            nc.vector.tensor_add(Vacc[:, :], Vacc[:, :], vt[:, :])
                    for g in range(4):
                        nc.sync.dma_start(out=V4[32 * g:32 * g + 32, :], in_=Vacc[:, :])

            # ======== pass 1: s0 = sum_i u_hat / 64 ========
            sacc = pa.tile([B, OD2], f32, tag="sacc", name="sacc0")
            for tau in range(NJ):
                for ch in range(4):
                    nc.tensor.matmul(
                        sacc[0:B, ch * 512:(ch + 1) * 512],
                        lhsT=x2t[:, tau * B:(tau + 1) * B],
                        rhs=wl[:, tau * OD2 + ch * 512: tau * OD2 + (ch + 1) * 512],
                        start=(tau == 0), stop=(tau == NJ - 1),
                        tile_position=(0, 0))
            s_sb = allreduce_s(0, sacc)
            squash(0, s_sb)

            # ======== passes 2..3: fused agreement/softmax/s ========
            for t in range(1, NUM_ROUTES):
                sacc = pa.tile([B, OD2], f32, tag="sacc", name=f"sacc{t}")
                for q in range(2 * NJ):
                    jj, s_ = divmod(q, 2)
                    uhsb = wp.tile([128, OD2], bf16, tag="uhb", name=f"uhsb{t}_{q}", bufs=4)
                    for h in range(2):
                        uh = pu.tile([128, 1024], f32, tag="uh", name=f"uh{t}_{q}_{h}")
                        for ch in range(2):
                            col0 = jj * OD2 + h * 1024 + ch * 512
                            for r in range(4):
                                nc.tensor.matmul(
                                    uh[32 * r:32 * r + 32, ch * 512:(ch + 1) * 512],
                                    lhsT=xs[s_][32 * r:32 * r + 32, jj * B:(jj + 1) * B],
                                    rhs=wl[32 * r:32 * r + 32, col0:col0 + 512],
                                    start=True, stop=True,
                                    tile_position=(32 * r, 32 * r),
                                )
                        # scalar engine evacuates u_hat (f32->bf16): frees
                        # the PSUM half so the PE starts the next one.
                        nc.scalar.copy(uhsb[:, h * 1024:(h + 1) * 1024], uh[:, :])
                    # agreement: tmp = u_hat * V (2x), then halving tree
                    # over d (all contiguous bf16 2x adds)
                    tmp = wp.tile([128, OD2], bf16, tag="tmp", name=f"tmp{t}_{q}")
                    nc.vector.tensor_mul(tmp[:, :], uhsb[:, :], V4[:, :])
                    # tree level 0 on gpsimd (parallel sidecar), rest on DVE
                    trL = wp.tile([128, OD2 // 2], bf16, tag="trL", name=f"trL{t}_{q}")
                    with nc.allow_low_precision(reason="bf16 agreement tree"):
                        nc.gpsimd.tensor_tensor(out=trL[:, :], in0=tmp[:, :1024],
                                                in1=tmp[:, 1024:], op=ALU.add)
                        w = OD2 // 2
                        for lv in range(3):
                            w //= 2
                            nc.vector.tensor_tensor(out=trL[:, :w], in0=trL[:, :w],
                                                    in1=trL[:, w:2 * w], op=ALU.add)
                    agr = sp.tile([128, OC], f32, tag="agr")
                    nc.vector.tensor_tensor(out=agr[:, :], in0=trL[:, :OC],
                                            in1=trL[:, OC:2 * OC], op=ALU.add)
                    eB = sp.tile([128, OC], bf16, tag="eB")
                    Zs = sp.tile([128, 1], f32, tag="Zs")
                    nc.scalar.activation(eB[:, :], agr[:, :], ACTF.Exp,
                                         accum_out=Zs[:, :])
                    rZ = sp.tile([128, 1], f32, tag="rZ")
                    nc.vector.reciprocal(rZ[:, :], Zs[:, :])
                    eB2 = sp.tile([128, OC], bf16, tag="eB2")
                    nc.scalar.mul(eB2[:, :], eB[:, :], rZ[:, :])
                    # c-apply: ONE tensor_tensor, eB2 broadcast over outer d
                    tmp2 = wp.tile([128, OD2], bf16, tag="tmp2b", name=f"tmp2b{t}_{q}")
                    nc.vector.tensor_tensor(
                        out=tmp2[:, :].rearrange("p (d o) -> p d o", o=OC),
                        in0=uhsb[:, :].rearrange("p (d o) -> p d o", o=OC),
                        in1=eB2[:, :].unsqueeze(1).broadcast_to([128, OD, OC]),
                        op=ALU.mult)
                    for ch in range(4):
                        nc.tensor.matmul(
                            sacc[0:B, ch * 512:(ch + 1) * 512], lhsT=sel1[:, :],
                            rhs=tmp2[:, ch * 512:(ch + 1) * 512],
                            start=(q == 0), stop=(q == 2 * NJ - 1),
                            tile_position=(0, 0))
                s_sb = allreduce_s(t, sacc)
                squash(t, s_sb)

    nc.compile()
    return nc


def _host_inputs(x, W):
    """Build per-core input maps (host-side relayout, not device time)."""
    W0 = np.asarray(W)[0]                       # [IC, OC, OD, KD]
    x = np.asarray(x)                           # [B, IC, KD]
    in_maps = []
    sel1 = np.zeros((128, 32), np.float32)
    for p in range(128):
        sel1[p, p % 32] = 1.0
    for c in range(NCORES):
        # W layout: partition 16*i8 + k, col tau*2048 + (d*64 + o)  [(d,o)!]
        Wc = W0[c * ICC:(c + 1) * ICC].reshape(NJ, 8, OC, OD, KD)   # [tau, i8, o, d, k]
        WL = np.ascontiguousarray(Wc.transpose(1, 4, 0, 3, 2)       # [i8, k, tau, d, o]
                                  ).reshape(128, NJ * OD2)
        xc = x[:, c * ICC:(c + 1) * ICC, :].reshape(B, NJ, 8, KD)   # [b, tau, i8, k]
        xss = []
        for s in range(2):
            Xs = np.zeros((4, 2, KD, NJ, B), np.float32)            # [r, s', k, tau, b]
            Xs[:, s] = xc[:, :, s::2].transpose(2, 3, 1, 0)         # [r, k, tau, b]
            xss.append(Xs.reshape(128, NJ * B))
        X2 = (np.ascontiguousarray(xc.transpose(2, 3, 1, 0))        # [i8, k, tau, b]
              .reshape(128, NJ * B) / float(OC))
        in_maps.append({
            "WL": WL.astype(ml_dtypes.bfloat16),
            "xS0": xss[0].astype(ml_dtypes.bfloat16),
            "xS1": xss[1].astype(ml_dtypes.bfloat16),
            "SEL1": sel1.astype(ml_dtypes.bfloat16),
            "X2": X2.astype(ml_dtypes.bfloat16),
        })
    return in_maps


def kernel(x, W, _want_trace=False):
    from concourse.bass_utils import run_bass_kernel_spmd

    if "nc" not in _CACHE:
        _CACHE["nc"] = _build_program()
    nc = _CACHE["nc"]
    in_maps = _host_inputs(x, W)
    res = run_bass_kernel_spmd(nc, in_maps, core_ids=list(range(NCORES)),
                               trace=_want_trace)
    _CACHE["last_result"] = res
    out = np.asarray(res.results[0]["v_out"], np.float32)
    # device output is [B, (d, o)] -> reorder to [B, OC, OD]
    return np.ascontiguousarray(out.reshape(B, OD, OC).transpose(0, 2, 1))


# revision 15
# speedup vs baseline: 1.8019x; 1.2716x over previous
"""CapsuleLayer dynamic-routing kernel for 8 Trainium2 NeuronCores.

Problem: x[32, 2048, 16], W[1, 2048, 64, 32, 16] -> v[32, 64, 32]
  u_hat = einsum('iodk,bik->biod', W[0], x)
  3 routing iterations (softmax over out_caps, squash over out_dim).

Sharding: in_caps (i) split 8 ways (256/core), W shard SBUF-resident bf16.

v2 layout: u_hat columns are (d, o)-ordered [col = d*64 + o].  This makes
every hot DVE op eligible for the 2x/4x perf modes:
  - V-mul: plain bf16*bf16 tensor_tensor (2x)
  - agreement d-reduce: 5 contiguous halving adds (bf16 2x) instead of a
    1x tensor_reduce
  - c-apply: rZ folded into eB ([128,64] tensor_scalar, 4x), then ONE
    tensor_tensor with eB broadcast on the OUTER (d) dim so the innermost
    AP stays packed (2x)
PSUM: u_hat quad is built in two [128,1024] halves (2 banks each, double
buffered) + s accumulator [32,2048] (4 banks) = 8 banks total, so the PE
can run ahead of the scalar-engine evacuation.

Routing state trick: b_ij(t) = sum_d u_hat * (v_0+...+v_{t-1}), so no
b_ij state is carried - only the accumulated V (bf16 [128,2048], x4
partition-group replication).
"""

import numpy as np
import ml_dtypes

B, IC, KD, OC, OD = 32, 2048, 16, 64, 32     # batch, in_caps, in_dim, out_caps, out_dim
NCORES = 8
ICC = IC // NCORES                            # 256 in_caps per core
NJ = ICC // 8                                 # 32 j-blocks (8 i per block)
OD2 = OC * OD                                 # 2048 flattened (d, o) columns
NUM_ROUTES = 3

_CACHE = {}


def _build_program():
    import concourse.bacc as bacc
    import concourse.tile as tile
    import concourse.mybir as mybir

    f32 = mybir.dt.float32
    bf16 = mybir.dt.bfloat16
    ALU = mybir.AluOpType
    ACTF = mybir.ActivationFunctionType

    nc = bacc.Bacc("TRN2", target_bir_lowering=False, debug=False, num_devices=NCORES)

    WL_d = nc.dram_tensor("WL", [128, NJ * OD2], bf16, kind="ExternalInput").ap()
    xS0_d = nc.dram_tensor("xS0", [128, NJ * B], bf16, kind="ExternalInput").ap()
    xS1_d = nc.dram_tensor("xS1", [128, NJ * B], bf16, kind="ExternalInput").ap()
    SEL1_d = nc.dram_tensor("SEL1", [128, 32], bf16, kind="ExternalInput").ap()
    X2_d = nc.dram_tensor("X2", [128, NJ * B], bf16, kind="ExternalInput").ap()
    vout_d = nc.dram_tensor("v_out", [B, OD2], f32, kind="ExternalOutput").ap()

    with tile.TileContext(nc) as tc:
        with (
            tc.tile_pool(name="const", bufs=1) as cp,
            tc.tile_pool(name="work", bufs=2) as wp,
            tc.tile_pool(name="small", bufs=2) as sp,
            tc.tile_pool(name="psacc", bufs=1, space="PSUM") as pa,
            tc.tile_pool(name="psuh", bufs=2, space="PSUM") as pu,
            tc.tile_pool(name="dram", bufs=1, space="DRAM") as dp,
        ):
            # ---- resident inputs (small stationaries first: pass 1 needs
            # x2t before any W chunk is useful) ----
            xs = [cp.tile([128, NJ * B], bf16, tag=f"xs{s}", name=f"xs{s}") for s in range(2)]
            sel1 = cp.tile([128, 32], bf16, tag="sel1")
            x2t = cp.tile([128, NJ * B], bf16, tag="x2t")
            nc.sync.dma_start(out=x2t[:, :], in_=X2_d[:, :])
            nc.sync.dma_start(out=xs[0][:, :], in_=xS0_d[:, :])
            nc.sync.dma_start(out=xs[1][:, :], in_=xS1_d[:, :])
            nc.sync.dma_start(out=sel1[:, :], in_=SEL1_d[:, :])
            wl = cp.tile([128, NJ * OD2], bf16, tag="wl")
            for blk in range(8):
                w = NJ * OD2 // 8
                nc.sync.dma_start(out=wl[:, blk * w:(blk + 1) * w],
                                  in_=WL_d[:, blk * w:(blk + 1) * w])

            # ---- persistent state ----
            V4 = cp.tile([128, OD2], bf16, tag="V4")   # V replicated x4 part-groups
            Vacc = cp.tile([B, OD2], bf16, tag="Vacc")  # running sum of v_t (bf16)

            ar_dt = [bf16, bf16, f32]
            ar_in = [dp.tile([B, OD2], ar_dt[t], tag=f"ari{t}", name=f"ari{t}")
                     for t in range(NUM_ROUTES)]
            ar_out = [dp.tile([B, OD2], ar_dt[t], tag=f"aro{t}", name=f"aro{t}",
                              addr_space="Shared")
                      for t in range(NUM_ROUTES)]

            def allreduce_s(t, src_psum):
                """Evacuate s (psum [32, 2048]) -> allreduce -> s_sb.
                Passes 0,1 run the wire in bf16 (half the bytes)."""
                s_sb = cp.tile([B, OD2], ar_dt[t], tag=f"ssb{t}", name=f"s_sb{t}")
                nc.scalar.copy(s_sb[:, :], src_psum[0:B, :])
                nc.sync.dma_start(out=ar_in[t][:, :], in_=s_sb[:, :])
                nc.gpsimd.collective_compute(
                    "AllReduce", ALU.add,
                    replica_groups=[list(range(NCORES))],
                    ins=[ar_in[t].opt()],
                    outs=[ar_out[t].opt()],
                )
                nc.sync.dma_start(out=s_sb[:, :], in_=ar_out[t][:, :])
                return s_sb

            def squash(t, s_sb):
                """v_t = squash(s_sb), (d,o) layout.  t<2: Vacc += v_t,
                V4 <- bf16 replicate(Vacc).  t==2: DMA v_t to output."""
                sq = wp.tile([B, OD2], f32, tag="sqv", name=f"sq{t}", bufs=1)
                nc.scalar.activation(sq[:, :], s_sb[:, :], ACTF.Square)
                # n2[b, o] = sum_d s^2 : in-place halving tree over outer d
                w = OD2
                for lv in range(5):
                    w //= 2
                    nc.vector.tensor_tensor(out=sq[:, :w], in0=sq[:, :w],
                                            in1=sq[:, w:2 * w], op=ALU.add)
                n2 = sp.tile([B, OC], f32, tag="n2")
                nc.vector.tensor_copy(n2[:, :], sq[:, :OC])
                r0 = sp.tile([B, OC], f32, tag="r0")
                nc.scalar.activation(r0[:, :], n2[:, :], ACTF.Sqrt)
                # Newton polish: n = 0.5 * (r0 + n2 / r0)
                t1 = sp.tile([B, OC], f32, tag="t1")
                nc.vector.reciprocal(t1[:, :], r0[:, :])
                nc.vector.tensor_mul(t1[:, :], t1[:, :], n2[:, :])
                t2 = sp.tile([B, OC], f32, tag="t2")
                nc.vector.tensor_add(t2[:, :], t1[:, :], r0[:, :])
                nn = sp.tile([B, OC], f32, tag="nn")
                nc.vector.tensor_scalar_mul(nn[:, :], t2[:, :], 0.5)   # |s|
                den = sp.tile([B, OC], f32, tag="den")
                nc.vector.tensor_scalar_add(den[:, :], n2[:, :], 1.0)
                rec = sp.tile([B, OC], f32, tag="rec")
                nc.vector.reciprocal(rec[:, :], den[:, :])
                qq = sp.tile([B, OC], f32, tag="qq")
                nc.vector.tensor_mul(qq[:, :], nn[:, :], rec[:, :])  # |s|/(1+|s|^2)
                vt = wp.tile([B, OD2], f32, tag="sqv", name=f"vt{t}", bufs=1)
                # v = s * qq broadcast over the OUTER d dim (innermost o packed)
                nc.vector.tensor_tensor(
                    out=vt[:, :].rearrange("p (d o) -> p d o", o=OC),
                    in0=s_sb[:, :].rearrange("p (d o) -> p d o", o=OC),
                    in1=qq[:, :].unsqueeze(1).broadcast_to([B, OD, OC]),
                    op=ALU.mult)
                if t == NUM_ROUTES - 1:
                    nc.sync.dma_start(out=vout_d[:, :], in_=vt[:, :])
                else:
                    if t == 0:
                        nc.vector.tensor_copy(Vacc[:, :], vt[:, :])
                    else:
                        with nc.allow_low_precision(reason="V accum in bf16"):
                            nc.vector.tensor_add(Vacc[:, :], Vacc[:, :], vt[:, :])
                    for g in range(4):
                        nc.sync.dma_start(out=V4[32 * g:32 * g + 32, :], in_=Vacc[:, :])

            # ======== pass 1: s0 = sum_i u_hat / 64 ========
            sacc = pa.tile([B, OD2], f32, tag="sacc", name="sacc0")
            for tau in range(NJ):
                for ch in range(4):
                    nc.tensor.matmul(
                        sacc[0:B, ch * 512:(ch + 1) * 512],
                        lhsT=x2t[:, tau * B:(tau + 1) * B],
                        rhs=wl[:, tau * OD2 + ch * 512: tau * OD2 + (ch + 1) * 512],
                        start=(tau == 0), stop=(tau == NJ - 1),
                        tile_position=(0, 0))
            s_sb = allreduce_s(0, sacc)
            squash(0, s_sb)

            # ======== passes 2..3: fused agreement/softmax/s ========
            for t in range(1, NUM_ROUTES):
                sacc = pa.tile([B, OD2], f32, tag="sacc", name=f"sacc{t}")

                def emit_capply(t, pq, uhsb_p, eB2_p):
                    """c-apply + sel-matmul for a prior quad pq (staggered so
                    it fills the softmax round-trip latency of quad pq+1)."""
                    tmp2 = wp.tile([128, OD2], bf16, tag="tmp2b", name=f"tmp2b{t}_{pq}")
                    nc.vector.tensor_tensor(
                        out=tmp2[:, :].rearrange("p (d o) -> p d o", o=OC),
                        in0=uhsb_p[:, :].rearrange("p (d o) -> p d o", o=OC),
                        in1=eB2_p[:, :].unsqueeze(1).broadcast_to([128, OD, OC]),
                        op=ALU.mult)
                    for ch in range(4):
                        nc.tensor.matmul(
                            sacc[0:B, ch * 512:(ch + 1) * 512], lhsT=sel1[:, :],
                            rhs=tmp2[:, ch * 512:(ch + 1) * 512],
                            start=(pq == 0), stop=(pq == 2 * NJ - 1),
                            tile_position=(0, 0))

                pending = None   # (pq, uhsb, eB2) awaiting c-apply
                for q in range(2 * NJ):
                    jj, s_ = divmod(q, 2)
                    uhsb = wp.tile([128, OD2], bf16, tag="uhb", name=f"uhsb{t}_{q}", bufs=4)
                    for h in range(2):
                        uh = pu.tile([128, 1024], f32, tag="uh", name=f"uh{t}_{q}_{h}")
                        for ch in range(2):
                            col0 = jj * OD2 + h * 1024 + ch * 512
                            for r in range(4):
                                nc.tensor.matmul(
                                    uh[32 * r:32 * r + 32, ch * 512:(ch + 1) * 512],
                                    lhsT=xs[s_][32 * r:32 * r + 32, jj * B:(jj + 1) * B],
                                    rhs=wl[32 * r:32 * r + 32, col0:col0 + 512],
                                    start=True, stop=True,
                                    tile_position=(32 * r, 32 * r),
                                )
                        # scalar engine evacuates u_hat (f32->bf16): frees
                        # the PSUM half so the PE starts the next one.
                        nc.scalar.copy(uhsb[:, h * 1024:(h + 1) * 1024], uh[:, :])
                    # agreement: tmp = u_hat * V (2x), then in-place halving
                    # tree over d (contiguous bf16 2x adds)
                    tmp = wp.tile([128, OD2], bf16, tag="tmp", name=f"tmp{t}_{q}")
                    nc.vector.tensor_mul(tmp[:, :], uhsb[:, :], V4[:, :])
                    w = OD2
                    with nc.allow_low_precision(reason="bf16 agreement tree"):
                        for lv in range(4):
                            w //= 2
                            nc.vector.tensor_tensor(out=tmp[:, :w], in0=tmp[:, :w],
                                                    in1=tmp[:, w:2 * w], op=ALU.add)
                    agr = sp.tile([128, OC], f32, tag="agr")
                    nc.vector.tensor_tensor(out=agr[:, :], in0=tmp[:, :OC],
                                            in1=tmp[:, OC:2 * OC], op=ALU.add)
                    # prior quad's c-apply fills this quad's softmax round-trip
                    if pending is not None:
                        emit_capply(t, *pending)
                    eB = sp.tile([128, OC], bf16, tag="eB")
                    Zs = sp.tile([128, 1], f32, tag="Zs")
                    nc.scalar.activation(eB[:, :], agr[:, :], ACTF.Exp,
                                         accum_out=Zs[:, :])
                    rZ = sp.tile([128, 1], f32, tag="rZ")
                    nc.vector.reciprocal(rZ[:, :], Zs[:, :])
                    eB2 = sp.tile([128, OC], bf16, tag="eB2")
                    nc.vector.tensor_scalar_mul(eB2[:, :], eB[:, :], rZ[:, :])
                    pending = (q, uhsb, eB2)
                emit_capply(t, *pending)
                s_sb = allreduce_s(t, sacc)
                squash(t, s_sb)

    nc.compile()
    return nc


def _host_inputs(x, W):
    """Build per-core input maps (host-side relayout, not device time)."""
    W0 = np.asarray(W)[0]                       # [IC, OC, OD, KD]
    x = np.asarray(x)                           # [B, IC, KD]
    in_maps = []
    sel1 = np.zeros((128, 32), np.float32)
    for p in range(128):
        sel1[p, p % 32] = 1.0
    for c in range(NCORES):
        # W layout: partition 16*i8 + k, col tau*2048 + (d*64 + o)  [(d,o)!]
        Wc = W0[c * ICC:(c + 1) * ICC].reshape(NJ, 8, OC, OD, KD)   # [tau, i8, o, d, k]
        WL = np.ascontiguousarray(Wc.transpose(1, 4, 0, 3, 2)       # [i8, k, tau, d, o]
                                  ).reshape(128, NJ * OD2)
        xc = x[:, c * ICC:(c + 1) * ICC, :].reshape(B, NJ, 8, KD)   # [b, tau, i8, k]
        xss = []
        for s in range(2):
            Xs = np.zeros((4, 2, KD, NJ, B), np.float32)            # [r, s', k, tau, b]
            Xs[:, s] = xc[:, :, s::2].transpose(2, 3, 1, 0)         # [r, k, tau, b]
            xss.append(Xs.reshape(128, NJ * B))
        X2 = (np.ascontiguousarray(xc.transpose(2, 3, 1, 0))        # [i8, k, tau, b]
              .reshape(128, NJ * B) / float(OC))
        in_maps.append({
            "WL": WL.astype(ml_dtypes.bfloat16),
            "xS0": xss[0].astype(ml_dtypes.bfloat16),
            "xS1": xss[1].astype(ml_dtypes.bfloat16),
            "SEL1": sel1.astype(ml_dtypes.bfloat16),
            "X2": X2.astype(ml_dtypes.bfloat16),
        })
    return in_maps


def kernel(x, W, _want_trace=False):
    from concourse.bass_utils import run_bass_kernel_spmd

    if "nc" not in _CACHE:
        _CACHE["nc"] = _build_program()
    nc = _CACHE["nc"]
    in_maps = _host_inputs(x, W)
    res = run_bass_kernel_spmd(nc, in_maps, core_ids=list(range(NCORES)),
                               trace=_want_trace)
    _CACHE["last_result"] = res
    out = np.asarray(res.results[0]["v_out"], np.float32)
    # device output is [B, (d, o)] -> reorder to [B, OC, OD]
    return np.ascontiguousarray(out.reshape(B, OD, OC).transpose(0, 2, 1))
